# revision 1
# baseline (speedup 1.0000x reference)
"""Chamfer distance (CDLoss) Trainium2 kernel.

Problem: prediction [4, 8192, 3], ground_truth [4, 8192, 3] (fp32).
For each batch: d2[n,m] = max(||p_n||^2 + ||g_m||^2 - 2 p.g, 0);
out[b] = sum_n min_m d2 / N + sum_m min_n d2 / M.

Strategy (8 NeuronCores): core c handles (batch = c//2, row-half = c%2),
i.e. a 4096 x 8192 slab of the distance matrix.

Device kernel per core (32 row blocks x 16 column tiles of [128, 512]):
  - Augmented-coordinate trick: ap[5, 4096] = [px, py, pz, ||p||^2, 1],
    ag[5, 8192] = [-2gx, -2gy, -2gz, 1, ||g||^2] so a single K=5 fp32
    matmul emits a [128, 512] tile of squared distances into one PSUM
    bank (PE time ~N cycles regardless of K).
  - VectorE tensor_reduce(min) per tile: exact fp32 row-min partial per
    (rowblock, coltile) into rowparts[128, 32*16].
  - ScalarE copy: PSUM -> SBUF cast to bf16 (the only other PSUM exit).
  - VectorE tensor_tensor(min) in bf16 (2x perf mode): running
    column-min buffer [128, 8192]. bf16 min is exact-monotone
    (min of rounded = rounded min), and the final sum of 8192 values
    concentrates the rounding to ~1e-5 relative.
Host: final tiny reductions (min over 128 partitions / 16 col tiles,
relu clamp, sums) in numpy. min-then-clamp == clamp-then-min, so the
relu of the reference moves to the host gather.
"""

import numpy as np

_B = 4
_N = 8192  # points per cloud
_HALF = _N // 2  # rows per core
_RB = _HALF // 128  # 32 row blocks
_GW = 512  # column group width (one PSUM bank)
_G = _N // _GW  # 16 column groups
_NCORES = 8

_CACHED_NC = None
_RUNNERS = {}


def _build_nc(repeat=1, variant="v1", gw=None, sbufs=6):
    import concourse.bacc as bacc
    import concourse.tile as tile
    from concourse import mybir

    f32 = mybir.dt.float32
    bf16 = mybir.dt.bfloat16

    gw = gw or _GW
    n_g = _N // gw
    n_mm = gw // 512  # matmuls (N<=512 fp32) per column group

    nc = bacc.Bacc("TRN2", target_bir_lowering=False, debug=False)

    ap_d = nc.dram_tensor("ap", [5, _HALF], f32, kind="ExternalInput")
    ag_d = nc.dram_tensor("ag", [5, _N], f32, kind="ExternalInput")
    rowparts_d = nc.dram_tensor(
        "rowparts", [128, _RB * n_g], f32, kind="ExternalOutput"
    )
    colmin_d = nc.dram_tensor("colmin", [128, _N], bf16, kind="ExternalOutput")

    BIG = 1.0e38

    with tile.TileContext(nc) as tc:
        with (
            tc.tile_pool(name="singles", bufs=1) as singles,
            tc.tile_pool(name="spool", bufs=sbufs) as spool,
            tc.tile_pool(name="rpool", bufs=3) as rpool,
            tc.tile_pool(name="psum", bufs=8 // n_mm, space="PSUM") as pp,
        ):
            ap_s = singles.tile([5, _HALF], f32)
            nc.sync.dma_start(out=ap_s[:], in_=ap_d[:])
            ag_s = singles.tile([5, _N], f32)
            nc.sync.dma_start(out=ag_s[:], in_=ag_d[:])

            colmin_s = singles.tile([128, _N], bf16)
            nc.vector.memset(colmin_s[:], BIG)
            rowparts_s = singles.tile([128, _RB * n_g], f32)
            if variant == "v3":
                # v3 only writes one rowparts column per row block; fill
                # the rest with BIG so the host-side min ignores them.
                nc.vector.memset(rowparts_s[:], BIG)

            def _body():
                for rb in range(_RB):
                    lhsT = ap_s[:, rb * 128 : (rb + 1) * 128]
                    if variant == "v3":
                        rowbuf = rpool.tile([128, gw], bf16, tag="rowbuf")
                        nc.vector.memset(rowbuf[:], BIG)
                    for g in range(n_g):
                        t = pp.tile([128, gw], f32, tag="t")
                        for j in range(n_mm):
                            c0 = g * gw + j * 512
                            nc.tensor.matmul(
                                t[:, j * 512 : (j + 1) * 512],
                                lhsT,
                                ag_s[:, c0 : c0 + 512],
                                start=True,
                                stop=True,
                            )
                        idx = rb * n_g + g
                        if variant == "v1":
                            # exact fp32 row minima (DVE, PSUM src, 1x)
                            nc.vector.tensor_reduce(
                                rowparts_s[:, idx : idx + 1],
                                t[:],
                                axis=mybir.AxisListType.X,
                                op=mybir.AluOpType.min,
                            )
                        # PSUM -> SBUF exit on ScalarE, cast to bf16
                        s = spool.tile([128, gw], bf16, tag="s")
                        nc.scalar.copy(s[:], t[:])
                        if variant == "v4":
                            # row minima from the bf16 copy (SBUF src is
                            # cheaper for DVE than fp32 PSUM)
                            nc.vector.tensor_reduce(
                                rowparts_s[:, idx : idx + 1],
                                s[:],
                                axis=mybir.AxisListType.X,
                                op=mybir.AluOpType.min,
                            )
                        if variant == "v3":
                            # row minima via cheap bf16 2x tensor_tensor
                            nc.vector.tensor_tensor(
                                rowbuf[:], rowbuf[:], s[:],
                                op=mybir.AluOpType.min,
                            )
                        # running column minima (bf16, DVE 2x mode)
                        cslice = colmin_s[:, g * gw : (g + 1) * gw]
                        nc.vector.tensor_tensor(
                            cslice, cslice, s[:], op=mybir.AluOpType.min
                        )
                    if variant == "v3":
                        nc.vector.tensor_reduce(
                            rowparts_s[:, rb * n_g : rb * n_g + 1],
                            rowbuf[:],
                            axis=mybir.AxisListType.X,
                            op=mybir.AluOpType.min,
                        )

            if repeat == 1:
                _body()
            else:
                # benchmark mode: body is idempotent (mins), repeat on-device
                with tc.For_i(0, repeat, 1):
                    _body()

            nc.sync.dma_start(out=rowparts_d[:], in_=rowparts_s[:])
            nc.sync.dma_start(out=colmin_d[:], in_=colmin_s[:])

    nc.compile()
    return nc


def _get_nc():
    global _CACHED_NC
    if _CACHED_NC is None:
        _CACHED_NC = _build_nc()
    return _CACHED_NC


def _prep_core_inputs(prediction, ground_truth):
    """Build per-core augmented matrices (host-side, fp32)."""
    in_maps = []
    for c in range(_NCORES):
        b, h = divmod(c, 2)
        p = np.asarray(prediction[b, h * _HALF : (h + 1) * _HALF], dtype=np.float32)
        g = np.asarray(ground_truth[b], dtype=np.float32)
        ap = np.empty((5, _HALF), dtype=np.float32)
        ap[0:3] = p.T
        ap[3] = (p * p).sum(axis=1, dtype=np.float32)
        ap[4] = 1.0
        ag = np.empty((5, _N), dtype=np.float32)
        ag[0:3] = (-2.0 * g).T
        ag[3] = 1.0
        ag[4] = (g * g).sum(axis=1, dtype=np.float32)
        in_maps.append({"ap": ap, "ag": ag})
    return in_maps


def _make_runner(nc, n_cores):
    """Build a cached jitted SPMD executor for `nc` (axon/PJRT path).

    Mirrors concourse.bass2jax.run_bass_via_pjrt but caches the jitted
    callable so repeat calls don't re-trace/re-compile.
    """
    import jax
    import numpy as _np
    from jax.sharding import Mesh, PartitionSpec
    from jax.experimental.shard_map import shard_map
    from concourse import mybir
    from concourse.bass2jax import (
        _bass_exec_p,
        install_neuronx_cc_hook,
        partition_id_tensor,
    )

    install_neuronx_cc_hook()

    partition_name = (
        nc.partition_id_tensor.name if nc.partition_id_tensor else None
    )
    in_names, out_names, out_avals, zero_shapes = [], [], [], []
    for alloc in nc.m.functions[0].allocations:
        if not isinstance(alloc, mybir.MemoryLocationSet):
            continue
        name = alloc.memorylocations[0].name
        if alloc.kind == "ExternalInput":
            if name == partition_name:
                continue
            in_names.append(name)
        elif alloc.kind == "ExternalOutput":
            shape = tuple(alloc.tensor_shape)
            dtype = mybir.dt.np(alloc.dtype)
            out_names.append(name)
            out_avals.append(jax.core.ShapedArray(shape, dtype))
            zero_shapes.append((shape, dtype))
    n_params = len(in_names)
    n_outs = len(out_names)
    all_names = in_names + out_names
    if partition_name is not None:
        all_names = all_names + [partition_name]
    donate = tuple(range(n_params, n_params + n_outs))

    def _body(*args):
        operands = list(args)
        if partition_name is not None:
            operands.append(partition_id_tensor())
        outs = _bass_exec_p.bind(
            *operands,
            out_avals=tuple(out_avals),
            in_names=tuple(all_names),
            out_names=tuple(out_names),
            lowering_input_output_aliases=(),
            sim_require_finite=True,
            sim_require_nnan=True,
            nc=nc,
        )
        return tuple(outs)

    devices = jax.devices()[:n_cores]
    mesh = Mesh(_np.asarray(devices), ("core",))
    sharded = jax.jit(
        shard_map(
            _body,
            mesh=mesh,
            in_specs=(PartitionSpec("core"),) * (n_params + n_outs),
            out_specs=(PartitionSpec("core"),) * n_outs,
            check_rep=False,
        ),
        donate_argnums=donate,
        keep_unused=True,
    )

    def run(in_maps):
        concat_in = [
            _np.concatenate([m[name] for m in in_maps], axis=0)
            for name in in_names
        ]
        concat_zeros = [
            _np.zeros((n_cores * s[0], *s[1:]), d) for (s, d) in zero_shapes
        ]
        out_arrs = sharded(*concat_in, *concat_zeros)
        return [
            {
                name: _np.asarray(out_arrs[i]).reshape(
                    n_cores, *out_avals[i].shape
                )[c]
                for i, name in enumerate(out_names)
            }
            for c in range(n_cores)
        ]

    return run


def _get_runner(nc, n_cores=_NCORES):
    key = id(nc)
    if key not in _RUNNERS:
        _RUNNERS[key] = _make_runner(nc, n_cores)
    return _RUNNERS[key]


def kernel(prediction, ground_truth):
    prediction = np.asarray(prediction, dtype=np.float32)
    ground_truth = np.asarray(ground_truth, dtype=np.float32)

    nc = _get_nc()
    in_maps = _prep_core_inputs(prediction, ground_truth)
    results = _get_runner(nc)(in_maps)

    out = np.zeros(_B, dtype=np.float32)
    for b in range(_B):
        dx = 0.0
        cms = []
        for h in range(2):
            r = results[2 * b + h]
            # rowparts[p, rb*G + g] = min over group g of row rb*128+p
            rp = r["rowparts"].reshape(128, _RB, _G).min(axis=2)  # [128, RB]
            dx += np.maximum(rp, 0.0).sum(dtype=np.float64)
            # colmin[p, j] = min over this core's row-blocks (partition p)
            cms.append(r["colmin"].astype(np.float32).min(axis=0))  # [N]
        cm = np.minimum(cms[0], cms[1])
        dy = np.maximum(cm, 0.0).sum(dtype=np.float64)
        out[b] = dx / _N + dy / _N
    return out



# revision 13
# speedup vs baseline: 3.1769x; 3.1769x over previous
"""Chamfer distance (CDLoss) Trainium2 kernel.

Problem: prediction [4, 8192, 3], ground_truth [4, 8192, 3] (fp32).
For each batch: d2[n,m] = max(||p_n||^2 + ||g_m||^2 - 2 p.g, 0);
out[b] = sum_n min_m d2 / N + sum_m min_n d2 / M.

Strategy (8 NeuronCores): core c handles (batch = c//2, row-half = c%2),
i.e. a 4096 x 8192 slab of the distance matrix.

Device kernel per core (32 row blocks x 4 column groups of [128, 2048]):
  - Augmented-coordinate trick: ap[5, 4096] = [px, py, pz, ||p||^2, 1],
    ag[5, 8192] = [-2gx, -2gy, -2gz, 1, ||g||^2] so K=5 fp32 matmuls
    emit [128, 512] tiles of squared distances; 4 of them fill a
    [128, 2048] PSUM tile (4 banks, double buffered).
  - ScalarE: one copy per group, PSUM fp32 -> SBUF bf16 (the PSUM exit).
  - VectorE, all bf16 in DVE 2x mode (tensor_reduce is 1x-only, so the
    hot loop avoids it):
      col direction: running column-min via tensor_tensor(min) per
        group; row-block 0 initializes colmin with tensor_copy (4x) so
        no [128, 8192] memset is needed.
      row direction: rowbuf = min(s0, s1) (one TT reads both tiles),
        fold s2, then ONE tensor_tensor_reduce on the last group fuses
        the final fold with the free-dim min-reduce: accum_out (fp32)
        = min over the row of bf16 distances. Output rowparts[128, 32].
bf16 min is exact-monotone (min of rounded == rounded min), so the only
error is bf16 rounding of each true min; the final sums average it to
~5e-5 relative.
Host: final tiny reductions (min over 128 partitions for colmin, relu
clamp, sums). min-then-clamp == clamp-then-min, so the reference's relu
moves to the host gather.
"""

import numpy as np

_B = 4
_N = 8192  # points per cloud
_HALF = _N // 2  # rows per core
_RB = _HALF // 128  # 32 row blocks
_GW = 2048  # column group width (4 PSUM banks)
_G = _N // _GW  # 4 column groups
_K = 13  # split-precision fp16 augmentation rows
_NCORES = 8

_CACHED_NC = None
_RUNNERS = {}

_BIG = 1.0e38


def _build_nc(repeat=1, variant="tt2r", gw=None):
    import concourse.bacc as bacc
    import concourse.tile as tile
    from concourse import mybir

    f32 = mybir.dt.float32
    f16 = mybir.dt.float16
    bf16 = mybir.dt.bfloat16
    MIN = mybir.AluOpType.min

    gw = gw or _GW
    n_g = _N // gw
    n_mm = gw // 512  # matmuls (N<=512 out per PSUM bank) per column group

    nc = bacc.Bacc("TRN2", target_bir_lowering=False, debug=False)

    ap_d = nc.dram_tensor("ap", [_K, _HALF], f16, kind="ExternalInput")
    ag_d = nc.dram_tensor("ag", [_K, _N], f16, kind="ExternalInput")
    rowparts_d = nc.dram_tensor("rowparts", [128, _RB], f32, kind="ExternalOutput")
    colmin_d = nc.dram_tensor("colmin", [128, _N], bf16, kind="ExternalOutput")

    with tile.TileContext(nc) as tc:
        with (
            tc.tile_pool(name="singles", bufs=1) as singles,
            tc.tile_pool(name="spool", bufs=8) as spool,
            tc.tile_pool(name="rpool", bufs=2) as rpool,
            tc.tile_pool(name="psum", bufs=8 // n_mm, space="PSUM") as pp,
        ):
            ap_s = singles.tile([_K, _HALF], f16)
            nc.sync.dma_start(out=ap_s[:], in_=ap_d[:])
            ag_s = singles.tile([_K, _N], f16)
            nc.sync.dma_start(out=ag_s[:], in_=ag_d[:])

            colmin_s = singles.tile([128, _N], bf16)
            rowparts_s = singles.tile([128, _RB], f32)
            if variant == "tt2j":
                junk = singles.tile([128, gw], bf16)
            else:
                junk = singles.tile([128, 1], bf16)
            if variant in ("mm", "mmc", "mmcol"):
                # ablation modes: outputs may be partially unwritten
                nc.vector.memset(colmin_s[:], _BIG)
                nc.vector.memset(rowparts_s[:], 0.0)

            def _body():
                for rb in range(_RB):
                    lhsT = ap_s[:, rb * 128 : (rb + 1) * 128]
                    stiles = []
                    rowbuf = rpool.tile([128, gw], bf16, tag="rowbuf")
                    for g in range(n_g):
                        t = pp.tile([128, gw], f32, tag="t")
                        for j in range(n_mm):
                            c0 = g * gw + j * 512
                            nc.tensor.matmul(
                                t[:, j * 512 : (j + 1) * 512],
                                lhsT,
                                ag_s[:, c0 : c0 + 512],
                                start=True,
                                stop=True,
                            )
                        if variant == "mm":
                            # ablation: PE only (WAW on the psum pool
                            # serializes reuse; no reader needed)
                            continue
                        # PSUM -> SBUF exit on ScalarE, cast to bf16
                        s = spool.tile([128, gw], bf16, tag=f"s{g}")
                        nc.scalar.copy(s[:], t[:])
                        stiles.append(s)
                        if variant == "mmc":
                            continue

                        # running column minima (bf16, DVE 2x mode)
                        cslice = colmin_s[:, g * gw : (g + 1) * gw]
                        if rb == 0:
                            nc.vector.tensor_copy(cslice, s[:])
                        else:
                            nc.vector.tensor_tensor(
                                cslice, cslice, s[:], op=MIN
                            )
                        if variant == "mmcol":
                            continue

                        # row minima: fold tiles into rowbuf (bf16 2x)
                        if g == 1:
                            nc.vector.tensor_tensor(
                                rowbuf[:], stiles[0][:], s[:], op=MIN
                            )
                        elif 1 < g < n_g - 1:
                            nc.vector.tensor_tensor(
                                rowbuf[:], rowbuf[:], s[:], op=MIN
                            )
                        elif g == n_g - 1:
                            # last fold fused with the free-dim min
                            # reduction (1x op, but replaces fold+reduce)
                            if variant in ("tt2", "tt2j"):
                                nc.vector.tensor_tensor_reduce(
                                    junk[:]
                                    if variant == "tt2j"
                                    else junk.broadcast_to((128, gw)),
                                    s[:],
                                    rowbuf[:],
                                    scale=1.0,
                                    scalar=_BIG,
                                    op0=MIN,
                                    op1=MIN,
                                    accum_out=rowparts_s[:, rb : rb + 1],
                                )
                            else:  # "tt2r": plain fold + 1x tensor_reduce
                                nc.vector.tensor_tensor(
                                    rowbuf[:], rowbuf[:], s[:], op=MIN
                                )
                                nc.vector.tensor_reduce(
                                    rowparts_s[:, rb : rb + 1],
                                    rowbuf[:],
                                    axis=mybir.AxisListType.X,
                                    op=MIN,
                                )

            if repeat == 1:
                _body()
            else:
                # benchmark mode: body is idempotent (mins), repeat on-device
                with tc.For_i(0, repeat, 1):
                    _body()

            nc.sync.dma_start(out=rowparts_d[:], in_=rowparts_s[:])
            nc.sync.dma_start(out=colmin_d[:], in_=colmin_s[:])

    nc.compile()
    return nc


def _get_nc():
    global _CACHED_NC
    if _CACHED_NC is None:
        _CACHED_NC = _build_nc()
    return _CACHED_NC


def _split16(x):
    """Split fp32 -> (hi, lo) fp16 pair with x ~= hi + lo to ~2^-24."""
    hi = x.astype(np.float16)
    lo = (x - hi.astype(np.float32)).astype(np.float16)
    return hi, lo


def _prep_core_inputs(prediction, ground_truth):
    """Build per-core K=13 split-precision fp16 augmented matrices.

    d2 = p^2 + g^2 - 2 p.g with every factor split into an fp16 hi/lo
    pair; fp16 x fp16 products are exact in the fp32 PSUM accumulate, so
    dropping only the lo*lo cross terms leaves ~2^-24 relative error.
    """
    in_maps = []
    for c in range(_NCORES):
        b, h = divmod(c, 2)
        p = np.asarray(prediction[b, h * _HALF : (h + 1) * _HALF], dtype=np.float32)
        g = np.asarray(ground_truth[b], dtype=np.float32)
        psq = (p * p).sum(axis=1, dtype=np.float32)
        gsq = (g * g).sum(axis=1, dtype=np.float32)
        s = -2.0 * g  # fold the -2 into the g side before splitting

        ap = np.empty((_K, _HALF), dtype=np.float16)
        ag = np.empty((_K, _N), dtype=np.float16)
        for d in range(3):
            p_hi, p_lo = _split16(p[:, d])
            s_hi, s_lo = _split16(s[:, d])
            ap[3 * d + 0] = p_hi
            ap[3 * d + 1] = p_hi
            ap[3 * d + 2] = p_lo
            ag[3 * d + 0] = s_hi
            ag[3 * d + 1] = s_lo
            ag[3 * d + 2] = s_hi
        ap[9], ap[10] = _split16(psq)
        ap[11] = 1.0
        ap[12] = 1.0
        ag[9] = 1.0
        ag[10] = 1.0
        ag[11], ag[12] = _split16(gsq)
        in_maps.append({"ap": ap, "ag": ag})
    return in_maps


def _make_runner(nc, n_cores):
    """Build a cached jitted SPMD executor for `nc` (axon/PJRT path).

    Mirrors concourse.bass2jax.run_bass_via_pjrt but caches the jitted
    callable so repeat calls don't re-trace/re-compile.
    """
    import jax
    import numpy as _np
    from jax.sharding import Mesh, PartitionSpec
    from jax.experimental.shard_map import shard_map
    from concourse import mybir
    from concourse.bass2jax import (
        _bass_exec_p,
        install_neuronx_cc_hook,
        partition_id_tensor,
    )

    install_neuronx_cc_hook()

    partition_name = (
        nc.partition_id_tensor.name if nc.partition_id_tensor else None
    )
    in_names, out_names, out_avals, zero_shapes = [], [], [], []
    for alloc in nc.m.functions[0].allocations:
        if not isinstance(alloc, mybir.MemoryLocationSet):
            continue
        name = alloc.memorylocations[0].name
        if alloc.kind == "ExternalInput":
            if name == partition_name:
                continue
            in_names.append(name)
        elif alloc.kind == "ExternalOutput":
            shape = tuple(alloc.tensor_shape)
            dtype = mybir.dt.np(alloc.dtype)
            out_names.append(name)
            out_avals.append(jax.core.ShapedArray(shape, dtype))
            zero_shapes.append((shape, dtype))
    n_params = len(in_names)
    n_outs = len(out_names)
    all_names = in_names + out_names
    if partition_name is not None:
        all_names = all_names + [partition_name]
    donate = tuple(range(n_params, n_params + n_outs))

    def _body(*args):
        operands = list(args)
        if partition_name is not None:
            operands.append(partition_id_tensor())
        outs = _bass_exec_p.bind(
            *operands,
            out_avals=tuple(out_avals),
            in_names=tuple(all_names),
            out_names=tuple(out_names),
            lowering_input_output_aliases=(),
            sim_require_finite=True,
            sim_require_nnan=True,
            nc=nc,
        )
        return tuple(outs)

    devices = jax.devices()[:n_cores]
    mesh = Mesh(_np.asarray(devices), ("core",))
    sharded = jax.jit(
        shard_map(
            _body,
            mesh=mesh,
            in_specs=(PartitionSpec("core"),) * (n_params + n_outs),
            out_specs=(PartitionSpec("core"),) * n_outs,
            check_rep=False,
        ),
        donate_argnums=donate,
        keep_unused=True,
    )

    def run(in_maps):
        concat_in = [
            _np.concatenate([m[name] for m in in_maps], axis=0)
            for name in in_names
        ]
        concat_zeros = [
            _np.zeros((n_cores * s[0], *s[1:]), d) for (s, d) in zero_shapes
        ]
        out_arrs = sharded(*concat_in, *concat_zeros)
        return [
            {
                name: _np.asarray(out_arrs[i]).reshape(
                    n_cores, *out_avals[i].shape
                )[c]
                for i, name in enumerate(out_names)
            }
            for c in range(n_cores)
        ]

    return run


def _get_runner(nc, n_cores=_NCORES):
    key = id(nc)
    if key not in _RUNNERS:
        _RUNNERS[key] = _make_runner(nc, n_cores)
    return _RUNNERS[key]


def kernel(prediction, ground_truth):
    prediction = np.asarray(prediction, dtype=np.float32)
    ground_truth = np.asarray(ground_truth, dtype=np.float32)

    nc = _get_nc()
    in_maps = _prep_core_inputs(prediction, ground_truth)
    results = _get_runner(nc)(in_maps)

    out = np.zeros(_B, dtype=np.float32)
    for b in range(_B):
        dx = 0.0
        cms = []
        for h in range(2):
            r = results[2 * b + h]
            # rowparts[p, rb] = min of row rb*128+p (bf16-rounded, fp32 accum)
            dx += np.maximum(r["rowparts"], 0.0).sum(dtype=np.float64)
            # colmin[p, j] = min over this core's row-blocks (partition p)
            cms.append(r["colmin"].astype(np.float32).min(axis=0))  # [N]
        cm = np.minimum(cms[0], cms[1])
        dy = np.maximum(cm, 0.0).sum(dtype=np.float64)
        out[b] = dx / _N + dy / _N
    return out


# revision 28
# speedup vs baseline: 3.3932x; 1.0681x over previous
"""Chamfer distance (CDLoss) Trainium2 kernel.

Problem: prediction [4, 8192, 3], ground_truth [4, 8192, 3] (fp32).
For each batch: d2[n,m] = max(||p_n||^2 + ||g_m||^2 - 2 p.g, 0);
out[b] = sum_n min_m d2 / N + sum_m min_n d2 / M.

Strategy (8 NeuronCores): core c handles (batch = c//2, row-half = c%2),
i.e. a 4096 x 8192 slab of the distance matrix.

Device kernel per core (32 row blocks x 4 column groups of [128, 2048]):
  - Split-precision fp16 matmul (PE fp32 runs at 4 cycles/row; fp16 at
    1): each factor of d2 = ||p||^2 + ||g||^2 - 2 p.g is split into an
    fp16 hi/lo pair, K=13 augmented rows. fp16 products accumulate
    exactly in fp32 PSUM; only lo*lo cross terms are dropped (~2^-24).
    Four [128, 512] matmuls fill a [128, 2048] PSUM tile (4 banks,
    double buffered). Measured ~478 ns/matmul -> PE ~245 us/core.
  - ScalarE: one copy per group, PSUM fp32 -> SBUF bf16. This is the
    PSUM exit and the critical engine: 1x rate, (2048+352)/1.2GHz
    ~2.1 us x 128 groups ~267 us/core. (Offloading copies to VectorE
    measured SLOWER overall: exit copies gate PSUM buffer reuse and
    stall the PE behind VectorE's deep queue.)
  - VectorE, all bf16 in DVE 2x mode (tensor_reduce is 1x-only, so the
    hot loop avoids it; ~0.9-1.1 us per [128,2048] op, ~230 us/core):
      col direction: running column-min via tensor_tensor(min) per
        group; row-block 0 initializes colmin with tensor_copy (4x) so
        no [128, 8192] memset is needed.
      row direction: rowbuf = min(s0, s1) (one TT reads both tiles),
        fold s2, fold s3, then halve with 2x TTs down to 512 wide and
        finish with one short 1x tensor_reduce into rowparts[128, 32].
bf16 min is exact-monotone (min of rounded == rounded min), so the only
error is bf16 rounding of each true min; the final sums average it to
~4e-5 relative.
Host: final tiny reductions (min over 128 partitions for colmin, relu
clamp, sums). min-then-clamp == clamp-then-min, so the reference's relu
moves to the host gather.

Rejected experiments (measured on HW): tensor_tensor_reduce fusion
(1x-only AND crashes the exec unit), fp32r matmul (unvalidated
numerics), 2048-wide single matmul (sim rejects multi-bank out),
ldweights elision (no gain), staggered_reset loop (breaks runtime),
DVE/DMA exit offload (PSUM coupling / no PSUM DMA source).
"""

import numpy as np

_B = 4
_N = 8192  # points per cloud
_HALF = _N // 2  # rows per core
_RB = _HALF // 128  # 32 row blocks
_GW = 2048  # column group width (4 PSUM banks)
_G = _N // _GW  # 4 column groups
_K = 13  # split-precision fp16 augmentation rows
_NCORES = 8

_CACHED_NC = None
_RUNNERS = {}

_BIG = 1.0e38


def _build_nc(repeat=1, variant="tt2r", gw=None, loop_mode="plain"):
    import concourse.bacc as bacc
    import concourse.tile as tile
    from concourse import mybir

    f32 = mybir.dt.float32
    f16 = mybir.dt.float16
    bf16 = mybir.dt.bfloat16
    MIN = mybir.AluOpType.min

    gw = gw or _GW
    n_g = _N // gw
    n_mm = gw // 512  # matmuls (N<=512 out per PSUM bank) per column group

    nc = bacc.Bacc("TRN2", target_bir_lowering=False, debug=False)

    ap_d = nc.dram_tensor("ap", [_K, _HALF], f16, kind="ExternalInput")
    ag_d = nc.dram_tensor("ag", [_K, _N], f16, kind="ExternalInput")
    rowparts_d = nc.dram_tensor("rowparts", [128, _RB], f32, kind="ExternalOutput")
    colmin_d = nc.dram_tensor("colmin", [128, _N], bf16, kind="ExternalOutput")

    with tile.TileContext(nc) as tc:
        with (
            tc.tile_pool(name="singles", bufs=1) as singles,
            tc.tile_pool(name="spool", bufs=8) as spool,
            tc.tile_pool(name="rpool", bufs=2) as rpool,
            tc.tile_pool(name="psum", bufs=8 // n_mm, space="PSUM") as pp,
        ):
            ap_s = singles.tile([_K, _HALF], f16)
            nc.sync.dma_start(out=ap_s[:], in_=ap_d[:])
            ag_s = singles.tile([_K, _N], f16)
            nc.sync.dma_start(out=ag_s[:], in_=ag_d[:])

            colmin_s = singles.tile([128, _N], bf16)
            rowparts_s = singles.tile([128, _RB], f32)
            if variant == "tt2j":
                junk = singles.tile([128, gw], bf16)
            else:
                junk = singles.tile([128, 1], bf16)
            if variant in ("mm", "mmc", "mmcol", "acto", "dveo"):
                # ablation modes: outputs may be partially unwritten
                nc.vector.memset(colmin_s[:], _BIG)
                nc.vector.memset(rowparts_s[:], 0.0)

            def _acto_body():
                # Act-only calibration: 128 copies from one static psum tile
                t0 = pp.tile([128, gw], f32, tag="t")
                nc.tensor.matmul(
                    t0[:, :512], ap_s[:, :128], ag_s[:, :512],
                    start=True, stop=True,
                )

                def body():
                    for i in range(_RB * n_g):
                        s = spool.tile([128, gw], bf16, tag="s0")
                        nc.scalar.copy(s[:], t0[:])
                        if i % 8 == 0:
                            nc.vector.tensor_tensor(
                                colmin_s[:, :64], colmin_s[:, :64],
                                s[:, :64], op=MIN,
                            )
                return body

            def _dveo_body():
                # DVE-only calibration: 128 in-place col TTs from a static s
                t0 = pp.tile([128, gw], f32, tag="t")
                nc.tensor.matmul(
                    t0[:, :512], ap_s[:, :128], ag_s[:, :512],
                    start=True, stop=True,
                )
                s0 = singles.tile([128, gw], bf16)
                nc.scalar.copy(s0[:], t0[:])

                def body():
                    for i in range(_RB * n_g):
                        cslice = colmin_s[:, (i % n_g) * gw : (i % n_g + 1) * gw]
                        nc.vector.tensor_tensor(cslice, cslice, s0[:], op=MIN)
                return body

            def _body():
                for rb in range(_RB):
                    lhsT = ap_s[:, rb * 128 : (rb + 1) * 128]
                    stiles = []
                    rowbuf = rpool.tile([128, gw], bf16, tag="rowbuf")
                    for g in range(n_g):
                        t = pp.tile([128, gw], f32, tag="t")
                        if variant == "tt5":
                            # single wide matmul spanning all banks
                            nc.tensor.matmul(
                                t[:],
                                lhsT,
                                ag_s[:, g * gw : (g + 1) * gw],
                                start=True,
                                stop=True,
                            )
                        else:
                            for j in range(n_mm):
                                c0 = g * gw + j * 512
                                mm = nc.tensor.matmul(
                                    t[:, j * 512 : (j + 1) * 512],
                                    lhsT,
                                    ag_s[:, c0 : c0 + 512],
                                    start=True,
                                    stop=True,
                                )
                                if variant == "tt2rl" and (g, j) != (0, 0):
                                    # lhsT identical for the whole row block:
                                    # skip reloading PE stationary weights
                                    mm.ins.ldweights = False
                        if variant == "mm":
                            # ablation: PE only (WAW on the psum pool
                            # serializes reuse; no reader needed)
                            continue
                        # PSUM -> SBUF exit, cast to bf16. ScalarE carries
                        # most copies; in tt3, DVE (which has slack) takes
                        # every 8th to debottleneck ScalarE.
                        s = spool.tile([128, gw], bf16, tag=f"s{g}")
                        if variant == "tt3" and (rb * n_g + g) % 8 == 4:
                            nc.vector.tensor_copy(s[:], t[:])
                        else:
                            nc.scalar.copy(s[:], t[:])
                        stiles.append(s)
                        if variant == "mmc":
                            continue

                        # running column minima (bf16, DVE 2x mode)
                        cslice = colmin_s[:, g * gw : (g + 1) * gw]
                        if rb == 0:
                            nc.vector.tensor_copy(cslice, s[:])
                        else:
                            nc.vector.tensor_tensor(
                                cslice, cslice, s[:], op=MIN
                            )
                        if variant == "mmcol":
                            continue

                        # row minima: fold tiles into rowbuf (bf16 2x)
                        if g == 1:
                            nc.vector.tensor_tensor(
                                rowbuf[:], stiles[0][:], s[:], op=MIN
                            )
                        elif 1 < g < n_g - 1:
                            nc.vector.tensor_tensor(
                                rowbuf[:], rowbuf[:], s[:], op=MIN
                            )
                        elif g == n_g - 1:
                            # last fold fused with the free-dim min
                            # reduction (1x op, but replaces fold+reduce)
                            if variant in ("tt2", "tt2j"):
                                nc.vector.tensor_tensor_reduce(
                                    junk[:]
                                    if variant == "tt2j"
                                    else junk.broadcast_to((128, gw)),
                                    s[:],
                                    rowbuf[:],
                                    scale=1.0,
                                    scalar=_BIG,
                                    op0=MIN,
                                    op1=MIN,
                                    accum_out=rowparts_s[:, rb : rb + 1],
                                )
                            else:  # "tt2r": plain fold, then reduce via
                                # 2x-mode halvings + a short 1x reduce
                                nc.vector.tensor_tensor(
                                    rowbuf[:], rowbuf[:], s[:], op=MIN
                                )
                                h = gw // 2
                                while h >= 512:
                                    nc.vector.tensor_tensor(
                                        rowbuf[:, :h],
                                        rowbuf[:, :h],
                                        rowbuf[:, h : 2 * h],
                                        op=MIN,
                                    )
                                    h //= 2
                                nc.vector.tensor_reduce(
                                    rowparts_s[:, rb : rb + 1],
                                    rowbuf[:, : 2 * h],
                                    axis=mybir.AxisListType.X,
                                    op=MIN,
                                )

            if variant == "acto":
                body = _acto_body()
            elif variant == "dveo":
                body = _dveo_body()
            else:
                body = _body

            if repeat == 1:
                body()
            else:
                # benchmark mode: body is idempotent (mins), repeat on-device
                if loop_mode == "fast":
                    import concourse.bass as bass

                    fi = tc.For_i(
                        0,
                        repeat,
                        1,
                        staggered_reset=True,
                        hint_engines=(mybir.EngineType.PE,),
                    )
                else:
                    fi = tc.For_i(0, repeat, 1)
                with fi:
                    body()

            nc.sync.dma_start(out=rowparts_d[:], in_=rowparts_s[:])
            nc.sync.dma_start(out=colmin_d[:], in_=colmin_s[:])

    nc.compile()
    return nc


def _get_nc():
    global _CACHED_NC
    if _CACHED_NC is None:
        _CACHED_NC = _build_nc()
    return _CACHED_NC


def _split16(x):
    """Split fp32 -> (hi, lo) fp16 pair with x ~= hi + lo to ~2^-24."""
    hi = x.astype(np.float16)
    lo = (x - hi.astype(np.float32)).astype(np.float16)
    return hi, lo


def _prep_core_inputs(prediction, ground_truth):
    """Build per-core K=13 split-precision fp16 augmented matrices.

    d2 = p^2 + g^2 - 2 p.g with every factor split into an fp16 hi/lo
    pair; fp16 x fp16 products are exact in the fp32 PSUM accumulate, so
    dropping only the lo*lo cross terms leaves ~2^-24 relative error.
    """
    in_maps = []
    for c in range(_NCORES):
        b, h = divmod(c, 2)
        p = np.asarray(prediction[b, h * _HALF : (h + 1) * _HALF], dtype=np.float32)
        g = np.asarray(ground_truth[b], dtype=np.float32)
        psq = (p * p).sum(axis=1, dtype=np.float32)
        gsq = (g * g).sum(axis=1, dtype=np.float32)
        s = -2.0 * g  # fold the -2 into the g side before splitting

        ap = np.empty((_K, _HALF), dtype=np.float16)
        ag = np.empty((_K, _N), dtype=np.float16)
        for d in range(3):
            p_hi, p_lo = _split16(p[:, d])
            s_hi, s_lo = _split16(s[:, d])
            ap[3 * d + 0] = p_hi
            ap[3 * d + 1] = p_hi
            ap[3 * d + 2] = p_lo
            ag[3 * d + 0] = s_hi
            ag[3 * d + 1] = s_lo
            ag[3 * d + 2] = s_hi
        ap[9], ap[10] = _split16(psq)
        ap[11] = 1.0
        ap[12] = 1.0
        ag[9] = 1.0
        ag[10] = 1.0
        ag[11], ag[12] = _split16(gsq)
        in_maps.append({"ap": ap, "ag": ag})
    return in_maps


def _make_runner(nc, n_cores):
    """Build a cached jitted SPMD executor for `nc` (axon/PJRT path).

    Mirrors concourse.bass2jax.run_bass_via_pjrt but caches the jitted
    callable so repeat calls don't re-trace/re-compile.
    """
    import jax
    import numpy as _np
    from jax.sharding import Mesh, PartitionSpec
    from jax.experimental.shard_map import shard_map
    from concourse import mybir
    from concourse.bass2jax import (
        _bass_exec_p,
        install_neuronx_cc_hook,
        partition_id_tensor,
    )

    install_neuronx_cc_hook()

    partition_name = (
        nc.partition_id_tensor.name if nc.partition_id_tensor else None
    )
    in_names, out_names, out_avals, zero_shapes = [], [], [], []
    for alloc in nc.m.functions[0].allocations:
        if not isinstance(alloc, mybir.MemoryLocationSet):
            continue
        name = alloc.memorylocations[0].name
        if alloc.kind == "ExternalInput":
            if name == partition_name:
                continue
            in_names.append(name)
        elif alloc.kind == "ExternalOutput":
            shape = tuple(alloc.tensor_shape)
            dtype = mybir.dt.np(alloc.dtype)
            out_names.append(name)
            out_avals.append(jax.core.ShapedArray(shape, dtype))
            zero_shapes.append((shape, dtype))
    n_params = len(in_names)
    n_outs = len(out_names)
    all_names = in_names + out_names
    if partition_name is not None:
        all_names = all_names + [partition_name]
    donate = tuple(range(n_params, n_params + n_outs))

    def _body(*args):
        operands = list(args)
        if partition_name is not None:
            operands.append(partition_id_tensor())
        outs = _bass_exec_p.bind(
            *operands,
            out_avals=tuple(out_avals),
            in_names=tuple(all_names),
            out_names=tuple(out_names),
            lowering_input_output_aliases=(),
            sim_require_finite=True,
            sim_require_nnan=True,
            nc=nc,
        )
        return tuple(outs)

    devices = jax.devices()[:n_cores]
    mesh = Mesh(_np.asarray(devices), ("core",))
    sharded = jax.jit(
        shard_map(
            _body,
            mesh=mesh,
            in_specs=(PartitionSpec("core"),) * (n_params + n_outs),
            out_specs=(PartitionSpec("core"),) * n_outs,
            check_rep=False,
        ),
        donate_argnums=donate,
        keep_unused=True,
    )

    def run(in_maps):
        concat_in = [
            _np.concatenate([m[name] for m in in_maps], axis=0)
            for name in in_names
        ]
        concat_zeros = [
            _np.zeros((n_cores * s[0], *s[1:]), d) for (s, d) in zero_shapes
        ]
        out_arrs = sharded(*concat_in, *concat_zeros)
        return [
            {
                name: _np.asarray(out_arrs[i]).reshape(
                    n_cores, *out_avals[i].shape
                )[c]
                for i, name in enumerate(out_names)
            }
            for c in range(n_cores)
        ]

    return run


def _get_runner(nc, n_cores=_NCORES):
    key = id(nc)
    if key not in _RUNNERS:
        _RUNNERS[key] = _make_runner(nc, n_cores)
    return _RUNNERS[key]


def kernel(prediction, ground_truth):
    prediction = np.asarray(prediction, dtype=np.float32)
    ground_truth = np.asarray(ground_truth, dtype=np.float32)

    nc = _get_nc()
    in_maps = _prep_core_inputs(prediction, ground_truth)
    results = _get_runner(nc)(in_maps)

    out = np.zeros(_B, dtype=np.float32)
    for b in range(_B):
        dx = 0.0
        cms = []
        for h in range(2):
            r = results[2 * b + h]
            # rowparts[p, rb] = min of row rb*128+p (bf16-rounded, fp32 accum)
            dx += np.maximum(r["rowparts"], 0.0).sum(dtype=np.float64)
            # colmin[p, j] = min over this core's row-blocks (partition p)
            cms.append(r["colmin"].astype(np.float32).min(axis=0))  # [N]
        cm = np.minimum(cms[0], cms[1])
        dy = np.maximum(cm, 0.0).sum(dtype=np.float64)
        out[b] = dx / _N + dy / _N
    return out
